# revision 3
# baseline (speedup 1.0000x reference)
"""GCNII layer (edge-weighted SpMM + BatchNorm + residual + linear blend + relu)
on 8 Trainium2 NeuronCores.

Strategy:
- Partition nodes (and edges by dst) across the 8 cores: core c owns nodes
  [6250c, 6250(c+1)).
- Per core, dst-sorted edges are grouped into 49 blocks of 128 nodes. The
  edge-weighted segment-sum is computed per block as a chain of 128x128
  matmuls: hT_block[d, v] += feat_tile[e, d].T-contraction  P_tile[e, v],
  where P_tile is the mask-scaled one-hot dst matrix built on-chip from an
  iota/is_equal compare.
- features[src] rows are fetched with batched dma_gather (1024 rows/call,
  rotated over 4 SWDGE queues). dma_gather indices are int16, so the feature
  table is split at row 32768 (lo/hi) and each block's edges are grouped by
  src-half, each half padded to whole 128-edge tiles.
- BatchNorm statistics: per-block row-sums (DVE reduce) and row-sums of
  squares (ACT Square+accum), followed by a [128, 2] f32 AllReduce across the
  8 cores.
- Everything runs transposed (d on partitions): BN apply is a per-partition
  tensor_scalar, the 128x128 linear is one stationary-weight matmul per
  512-column chunk, and the output is stored transposed and re-transposed on
  the host.
"""
import os
import numpy as np
import ml_dtypes

N = 50000
E = 800000
D = 128
P = 128
NCORES = 8
NPC = N // NCORES          # 6250 nodes per core
NBLK = (NPC + P - 1) // P  # 49 blocks (48 full + 106)
NSTRIP = NBLK * P          # 6272 strip columns
SPLIT = 32768              # int16 idx limit for dma_gather
NIDX = 1024                # rows per dma_gather call
TPC = NIDX // P            # 8 tiles per gather call
NQ = 4                     # SWDGE queues
ALPHA = 0.1
BETA = 0.5
BN_EPS = 1e-5

_last_exec_ns = None


def _wrap_idx(stream, ncall):
    """int idx stream [ncall*1024] -> dma_gather layout [128, 64*ncall] int16.

    Slot i of call g is read from idxs[i%16, i//16] (16-partition wrap,
    replicated 8x down the partitions).
    """
    out = np.empty((P, 64 * ncall), dtype=np.int16)
    a = stream.reshape(ncall, 64, 16)
    for g in range(ncall):
        t16 = a[g].T.astype(np.int16)          # [16, 64]
        out[:, 64 * g:64 * (g + 1)] = np.tile(t16, (8, 1))
    return out


def _preprocess(mask, src, dst):
    """Build per-core gather/index streams. Returns layout dict."""
    core = dst // NPC
    local = dst - core * NPC
    block = local // P
    dstpos = local - block * P
    hi = (src >= SPLIT).astype(np.int64)

    # sort edges by (core, block, hi) -- order within groups irrelevant
    key = ((core * NBLK + block) * 2 + hi)
    order = np.argsort(key, kind="stable")
    ks = key[order]
    src_s = src[order].astype(np.int64)
    dp_s = dstpos[order].astype(np.int64)
    mk_s = mask.reshape(-1)[order].astype(np.float32)

    # group boundaries for all (core, block, half) cells
    ncell = NCORES * NBLK * 2
    counts = np.bincount(ks, minlength=ncell).reshape(NCORES, NBLK, 2)
    starts = np.zeros(ncell + 1, dtype=np.int64)
    np.cumsum(counts.reshape(-1), out=starts[1:])

    # fixed tiles per (block, half): max over cores (SPMD same program)
    tiles = np.maximum((counts + P - 1) // P, 0).max(axis=0)  # [NBLK, 2]
    tiles[:, 0] = np.maximum(tiles[:, 0], 1)  # lo phase always present (copy)
    t_lo = tiles[:, 0]
    t_hi = tiles[:, 1]
    nt_lo = int(t_lo.sum())
    nt_hi = int(t_hi.sum())
    ncall_lo = (nt_lo * P + NIDX - 1) // NIDX
    ncall_hi = max((nt_hi * P + NIDX - 1) // NIDX, 0)
    nt_lo_pad = ncall_lo * TPC
    nt_hi_pad = ncall_hi * TPC
    nt = nt_lo_pad + nt_hi_pad

    # tile tables: for each (block, half): stream tile offset
    lo_off = np.zeros(NBLK, dtype=np.int64)
    np.cumsum(t_lo[:-1], out=lo_off[1:])
    hi_off = np.zeros(NBLK, dtype=np.int64)
    np.cumsum(t_hi[:-1], out=hi_off[1:])

    per_core = []
    for c in range(NCORES):
        slo = np.zeros(nt_lo_pad * P, dtype=np.int64)       # gather row idx
        shi = np.zeros(nt_hi_pad * P, dtype=np.int64)
        dpT = np.zeros(nt * P, dtype=np.float32)            # block-rel dst pos
        mkT = np.zeros(nt * P, dtype=np.float32)
        for b in range(NBLK):
            for h in (0, 1):
                cell = (c * NBLK + b) * 2 + h
                s0, s1 = starts[cell], starts[cell + 1]
                n = s1 - s0
                if h == 0:
                    base = lo_off[b] * P
                    slo[base:base + n] = src_s[s0:s1]
                else:
                    base = (nt_lo_pad + hi_off[b]) * P
                    shi[(hi_off[b] * P):(hi_off[b] * P) + n] = src_s[s0:s1] - SPLIT
                dpT[base:base + n] = dp_s[s0:s1]
                mkT[base:base + n] = mk_s[s0:s1]
        idx_lo = _wrap_idx(slo, ncall_lo)
        idx_hi = _wrap_idx(shi, ncall_hi) if ncall_hi else np.zeros((P, 0), np.int16)
        dpT = dpT.reshape(nt, P).T.astype(ml_dtypes.bfloat16)  # [128, nt]
        mkT = mkT.reshape(nt, P).T.astype(ml_dtypes.bfloat16)
        per_core.append((idx_lo, idx_hi, dpT, mkT))

    return {
        "t_lo": t_lo, "t_hi": t_hi, "lo_off": lo_off, "hi_off": hi_off,
        "nt_lo_pad": nt_lo_pad, "nt_hi_pad": nt_hi_pad, "nt": nt,
        "ncall_lo": ncall_lo, "ncall_hi": ncall_hi, "per_core": per_core,
    }


def _build_program(L):
    import concourse.bass as bass
    import concourse.mybir as mybir
    import concourse.tile as tile
    import concourse.bacc as bacc
    from contextlib import ExitStack

    bf16 = mybir.dt.bfloat16
    f32 = mybir.dt.float32

    nt = L["nt"]
    ncall_lo, ncall_hi = L["ncall_lo"], L["ncall_hi"]
    t_lo, t_hi = L["t_lo"], L["t_hi"]
    lo_off, hi_off = L["lo_off"], L["hi_off"]
    nt_lo_pad = L["nt_lo_pad"]

    # tile -> (block, pos_in_block, count_in_block, is_hi) map; -1 = padding
    tmap = [None] * nt
    for b in range(NBLK):
        for i in range(int(t_lo[b])):
            tmap[int(lo_off[b]) + i] = (b, i, int(t_lo[b]), 0)
        for i in range(int(t_hi[b])):
            tmap[nt_lo_pad + int(hi_off[b]) + i] = (b, i, int(t_hi[b]), 1)

    nc = bacc.Bacc("TRN2", num_swdge_queues=NQ)
    tab_lo = nc.dram_tensor("tab_lo", [SPLIT, D], bf16, kind="ExternalInput")
    tab_hi = nc.dram_tensor("tab_hi", [N - SPLIT, D], bf16, kind="ExternalInput")
    idx_lo_d = nc.dram_tensor("idx_lo", [P, 64 * ncall_lo], mybir.dt.int16,
                              kind="ExternalInput")
    if ncall_hi:
        idx_hi_d = nc.dram_tensor("idx_hi", [P, 64 * ncall_hi], mybir.dt.int16,
                                  kind="ExternalInput")
    dp_d = nc.dram_tensor("dpT", [P, nt], bf16, kind="ExternalInput")
    mk_d = nc.dram_tensor("mkT", [P, nt], bf16, kind="ExternalInput")
    x0_d = nc.dram_tensor("x0T", [P, NSTRIP], f32, kind="ExternalInput")
    wt_d = nc.dram_tensor("WT", [P, P], f32, kind="ExternalInput")
    gam_d = nc.dram_tensor("gammaP", [P, 1], f32, kind="ExternalInput")
    bet_d = nc.dram_tensor("betaP", [P, 1], f32, kind="ExternalInput")
    out_d = nc.dram_tensor("outT", [P, NPC], f32, kind="ExternalOutput")

    with ExitStack() as ctx:
        tc = ctx.enter_context(tile.TileContext(nc))
        const = ctx.enter_context(tc.tile_pool(name="const", bufs=1))
        gatp = ctx.enter_context(tc.tile_pool(name="gat", bufs=8))
        ptp = ctx.enter_context(tc.tile_pool(name="pt", bufs=4))
        psp = ctx.enter_context(tc.tile_pool(name="ps", bufs=4, space="PSUM"))
        pswp = ctx.enter_context(tc.tile_pool(name="psw", bufs=2, space="PSUM"))
        smallp = ctx.enter_context(tc.tile_pool(name="small", bufs=4))
        dram = ctx.enter_context(tc.tile_pool(name="dram", bufs=2, space="DRAM"))

        # ---- constants / inputs to SBUF ----
        iota_i = const.tile([P, NIDX], mybir.dt.int32)
        nc.gpsimd.iota(iota_i[:], pattern=[[0, TPC], [1, P]], base=0,
                       channel_multiplier=0)
        iota_b = const.tile([P, NIDX], bf16)
        nc.vector.tensor_copy(out=iota_b[:], in_=iota_i[:])

        idxlo_s = const.tile([P, 64 * ncall_lo], mybir.dt.int16)
        nc.sync.dma_start(out=idxlo_s[:], in_=idx_lo_d[:])
        if ncall_hi:
            idxhi_s = const.tile([P, 64 * ncall_hi], mybir.dt.int16)
            nc.sync.dma_start(out=idxhi_s[:], in_=idx_hi_d[:])
        dp_s = const.tile([P, nt], bf16)
        nc.sync.dma_start(out=dp_s[:], in_=dp_d[:])
        mk_s = const.tile([P, nt], bf16)
        nc.sync.dma_start(out=mk_s[:], in_=mk_d[:])
        x0_s = const.tile([P, NSTRIP], f32)
        nc.sync.dma_start(out=x0_s[:], in_=x0_d[:])
        wt_s = const.tile([P, P], f32)
        nc.sync.dma_start(out=wt_s[:], in_=wt_d[:])
        wt_b = const.tile([P, P], bf16)
        nc.vector.tensor_copy(out=wt_b[:], in_=wt_s[:])
        gam_s = const.tile([P, 1], f32)
        nc.sync.dma_start(out=gam_s[:], in_=gam_d[:])
        bet_s = const.tile([P, 1], f32)
        nc.sync.dma_start(out=bet_s[:], in_=bet_d[:])

        hT = const.tile([P, NSTRIP], f32)
        ssum = const.tile([P, NBLK], f32)
        ssq = const.tile([P, NBLK], f32)

        # ---- gather + aggregate streams ----
        psum_live = {}

        def flush_block(b, is_hi, ps):
            sl = hT[:, b * P:(b + 1) * P]
            if not is_hi:
                nc.vector.tensor_copy(out=sl, in_=ps[:])
            else:
                nc.vector.tensor_tensor(out=sl, in0=sl, in1=ps[:],
                                        op=mybir.AluOpType.add)
            if is_hi or t_hi[b] == 0:
                # block finished: BN partial stats
                sq = smallp.tile([P, P], f32, tag="sq")
                nc.scalar.activation(out=sq[:], in_=sl,
                                     func=mybir.ActivationFunctionType.Square,
                                     accum_out=ssq[:, b:b + 1])
                nc.vector.reduce_sum(out=ssum[:, b:b + 1], in_=sl,
                                     axis=mybir.AxisListType.X)

        def emit_stream(ncall, idx_s, tab, tile_base, is_hi):
            for g in range(ncall):
                gat = gatp.tile([P, TPC, P], bf16, tag="gat")
                nc.gpsimd.dma_gather(
                    gat[:], tab[:], idx_s[:, 64 * g:64 * (g + 1)],
                    NIDX, NIDX, P, queue_num=g % NQ,
                )
                t0 = tile_base + g * TPC
                # any real tiles in this call?
                if all(tmap[t0 + k] is None for k in range(TPC)
                       if t0 + k < nt):
                    continue
                pt = ptp.tile([P, NIDX], bf16, tag="pt")
                dpb = dp_s[:, t0:t0 + TPC].unsqueeze(2).to_broadcast([P, TPC, P])
                mkb = mk_s[:, t0:t0 + TPC].unsqueeze(2).to_broadcast([P, TPC, P])
                nc.vector.tensor_tensor(out=pt[:], in0=iota_b[:], in1=dpb,
                                        op=mybir.AluOpType.is_equal)
                nc.vector.tensor_tensor(out=pt[:], in0=pt[:], in1=mkb,
                                        op=mybir.AluOpType.mult)
                for k in range(TPC):
                    t = t0 + k
                    if t >= nt or tmap[t] is None:
                        continue
                    b, i, cnt, h = tmap[t]
                    if i == 0:
                        psum_live[(b, h)] = psp.tile([P, P], f32, tag="agg",
                                                     name=f"agg_{b}_{h}")
                    ps = psum_live[(b, h)]
                    nc.tensor.matmul(
                        out=ps[:],
                        lhsT=gat[:, k, :],
                        rhs=pt[:, k * P:(k + 1) * P],
                        start=(i == 0),
                        stop=(i == cnt - 1),
                    )
                    if i == cnt - 1:
                        flush_block(b, h, ps)
                        del psum_live[(b, h)]

        emit_stream(ncall_lo, idxlo_s, tab_lo, 0, 0)
        if ncall_hi:
            emit_stream(ncall_hi, idxhi_s, tab_hi, nt_lo_pad, 1)

        # ---- global BN stats (AllReduce of [128, 2]) ----
        stats2 = smallp.tile([P, 2], f32, tag="st2")
        nc.vector.reduce_sum(out=stats2[:, 0:1], in_=ssum[:, :NBLK],
                             axis=mybir.AxisListType.X)
        nc.vector.reduce_sum(out=stats2[:, 1:2], in_=ssq[:, :NBLK],
                             axis=mybir.AxisListType.X)
        cc_in = dram.tile([P, 2], f32)
        cc_out = dram.tile([P, 2], f32)
        nc.gpsimd.dma_start(out=cc_in[:], in_=stats2[:])
        nc.gpsimd.collective_compute(
            "AllReduce", mybir.AluOpType.add,
            ins=[cc_in.opt()], outs=[cc_out.opt()],
            replica_groups=[list(range(NCORES))],
        )
        statg = smallp.tile([P, 2], f32, tag="stg")
        nc.gpsimd.dma_start(out=statg[:], in_=cc_out[:])

        # ---- BN affine coefficients (per-partition [128, 1]) ----
        mean = smallp.tile([P, 1], f32, tag="c0")
        nc.vector.tensor_scalar(out=mean[:], in0=statg[:, 0:1], scalar1=1.0 / N,
                                scalar2=None, op0=mybir.AluOpType.mult)
        var = smallp.tile([P, 1], f32, tag="c1")
        # var = sq/N - mean^2
        nc.vector.tensor_scalar(out=var[:], in0=statg[:, 1:2], scalar1=1.0 / N,
                                scalar2=None, op0=mybir.AluOpType.mult)
        m2 = smallp.tile([P, 1], f32, tag="c2")
        nc.vector.tensor_tensor(out=m2[:], in0=mean[:], in1=mean[:],
                                op=mybir.AluOpType.mult)
        nc.vector.tensor_tensor(out=var[:], in0=var[:], in1=m2[:],
                                op=mybir.AluOpType.subtract)
        nc.vector.tensor_scalar(out=var[:], in0=var[:], scalar1=BN_EPS,
                                scalar2=None, op0=mybir.AluOpType.add)
        sd = smallp.tile([P, 1], f32, tag="c3")
        nc.scalar.activation(out=sd[:], in_=var[:],
                             func=mybir.ActivationFunctionType.Sqrt)
        rinv = smallp.tile([P, 1], f32, tag="c4")
        nc.vector.reciprocal(out=rinv[:], in_=sd[:])
        a9 = smallp.tile([P, 1], f32, tag="c5")
        nc.vector.tensor_tensor(out=a9[:], in0=gam_s[:], in1=rinv[:],
                                op=mybir.AluOpType.mult)
        nc.vector.tensor_scalar(out=a9[:], in0=a9[:], scalar1=1.0 - ALPHA,
                                scalar2=None, op0=mybir.AluOpType.mult)
        b9 = smallp.tile([P, 1], f32, tag="c6")
        # b9 = (1-alpha) * (beta - mean * gamma * rinv); note a9 already scaled
        nc.vector.tensor_tensor(out=b9[:], in0=mean[:], in1=a9[:],
                                op=mybir.AluOpType.mult)
        tmpb = smallp.tile([P, 1], f32, tag="c7")
        nc.vector.tensor_scalar(out=tmpb[:], in0=bet_s[:], scalar1=1.0 - ALPHA,
                                scalar2=None, op0=mybir.AluOpType.mult)
        nc.vector.tensor_tensor(out=b9[:], in0=tmpb[:], in1=b9[:],
                                op=mybir.AluOpType.subtract)

        # ---- BN apply + alpha residual (in place on hT) ----
        nc.vector.tensor_scalar(out=hT[:], in0=hT[:], scalar1=a9[:, :1],
                                scalar2=b9[:, :1], op0=mybir.AluOpType.mult,
                                op1=mybir.AluOpType.add)
        nc.scalar.mul(out=x0_s[:], in_=x0_s[:], mul=ALPHA)
        nc.vector.tensor_tensor(out=hT[:], in0=hT[:], in1=x0_s[:],
                                op=mybir.AluOpType.add)

        # ---- beta-blend with W matmul, relu, store ----
        h2b = const.tile([P, NSTRIP], bf16)
        nc.vector.tensor_copy(out=h2b[:], in_=hT[:])
        outS = const.tile([P, NSTRIP], f32)
        CH = 512
        for c0 in range(0, NSTRIP, CH):
            c1 = min(c0 + CH, NSTRIP)
            w = c1 - c0
            psw = pswp.tile([P, CH], f32, tag="psw")
            nc.tensor.matmul(out=psw[:, :w], lhsT=wt_b[:], rhs=h2b[:, c0:c1],
                             start=True, stop=True)
            tmp = smallp.tile([P, CH], f32, tag="blend")
            nc.vector.tensor_tensor(out=tmp[:, :w], in0=hT[:, c0:c1],
                                    in1=psw[:, :w], op=mybir.AluOpType.add)
            nc.scalar.activation(out=outS[:, c0:c1], in_=tmp[:, :w],
                                 func=mybir.ActivationFunctionType.Relu,
                                 scale=BETA)
        nc.sync.dma_start(out=out_d[:], in_=outS[:, :NPC])

    nc.compile()
    return nc


def kernel(features, initial_features, mask, W, gamma, beta_bn, src, dst):
    global _last_exec_ns
    features = np.asarray(features, dtype=np.float32)
    initial_features = np.asarray(initial_features, dtype=np.float32)
    mask = np.asarray(mask, dtype=np.float32)
    W = np.asarray(W, dtype=np.float32)
    gamma = np.asarray(gamma, dtype=np.float32)
    beta_bn = np.asarray(beta_bn, dtype=np.float32)
    src = np.asarray(src, dtype=np.int64)
    dst = np.asarray(dst, dtype=np.int64)

    L = _preprocess(mask, src, dst)
    nc = _build_program(L)

    from concourse.bass_utils import run_bass_kernel_spmd

    tab_lo = features[:SPLIT].astype(ml_dtypes.bfloat16)
    tab_hi = features[SPLIT:].astype(ml_dtypes.bfloat16)
    WT = np.ascontiguousarray(W.T).astype(np.float32)
    gammaP = gamma.reshape(P, 1).astype(np.float32)
    betaP = beta_bn.reshape(P, 1).astype(np.float32)

    in_maps = []
    for c in range(NCORES):
        idx_lo, idx_hi, dpT, mkT = L["per_core"][c]
        x0T = np.zeros((P, NSTRIP), dtype=np.float32)
        x0T[:, :NPC] = initial_features[c * NPC:(c + 1) * NPC].T
        m = {
            "tab_lo": tab_lo, "tab_hi": tab_hi,
            "idx_lo": idx_lo, "dpT": dpT, "mkT": mkT,
            "x0T": x0T, "WT": WT, "gammaP": gammaP, "betaP": betaP,
        }
        if L["ncall_hi"]:
            m["idx_hi"] = idx_hi
        in_maps.append(m)

    trace = os.environ.get("GCNII_TRACE", "0") == "1"
    if trace:
        import ntff_shim  # noqa: F401
    res = run_bass_kernel_spmd(nc, in_maps, list(range(NCORES)), trace=trace)
    _last_exec_ns = res.exec_time_ns

    out = np.empty((N, D), dtype=np.float32)
    for c in range(NCORES):
        out[c * NPC:(c + 1) * NPC] = res.results[c]["outT"][:, :NPC].T
    return out
